# revision 1
# baseline (speedup 1.0000x reference)
"""2-layer GCN (GCNConv -> ReLU -> BN -> GCNConv -> ReLU) on 8 trn2 NeuronCores.

Strategy (single SPMD program on all 8 cores):
  - One node sharding everywhere: nodes are ranked by degree (ascending) and
    dealt round-robin to cores, so every core's t-th 128-dst tile has nearly
    the same max indegree -> the padded per-tile slot count K is uniform
    across cores and padding stays ~5%.
  - x rows are host-permuted into table order; EVERY core computes the FULL
    layer-1 table h1*dinv itself (x @ W1 is cheap and DMA-bound, ~0.6 ms at
    full rate) -- this replicated compute eliminates the first AllGather
    (~1.7 ms of serial collective) entirely.
  - Normalization is folded into the tables: tables store h*dinv, so
    agg[d] = dinv[d] * sum_j table[src_j]; layer 1 takes the self-loop as an
    extra gather slot from the local table, layer 2 as a plain local f32 DMA
    from the core's own shard.
  - BN (eval mode) is folded into W2' = diag(s) @ W2 and c2 = t @ W2 on host,
    so layer-1 epilogue is: relu -> transpose (PE) -> @W2' -> *dinv.
  - Gathers use indirect_dma_start with one index per partition (128 rows /
    instruction at a measured 1.43 us/instruction engine floor -- the only
    HW-validated indexed-DMA form on this runtime; the MoE dma_gather /
    dma_scatter_add ucode library is absent here, and multi-offset indirect
    DMA mis-executes in ucode).
  - One AllGather (Shared-address DRAM, ~11x faster than Local) shares the
    layer-2 table across cores between the layers.

Host does only index/graph-structure preprocessing (sharding, degree counts,
padding layout, BN constant folding); all tensor math runs on device.
"""

import numpy as np

import concourse.bass as bass
import concourse.bacc as bacc
import concourse.mybir as mybir
import concourse.tile as tile
from concourse.bass_utils import run_bass_kernel_spmd

F32 = mybir.dt.float32
I32 = mybir.dt.int32

C = 8          # cores
P = 128        # partitions
H = 32         # hidden dim
D = 512        # input dim
BN_EPS = 1e-5


def _plan(n_nodes, edge_index):
    """Host-side graph preprocessing -> per-core index arrays + metadata."""
    src = np.asarray(edge_index[0], dtype=np.int64)
    dst = np.asarray(edge_index[1], dtype=np.int64)

    deg = np.bincount(dst, minlength=n_nodes).astype(np.float32) + 1.0
    dinv = (1.0 / np.sqrt(deg)).astype(np.float32)

    per = n_nodes // C                      # real rows per core (p1 + dst shards)
    SH = -(-per // 512) * 512               # shard rows, multiple of 512
    # tiles of 128 dsts; only tiles covering real rows are processed
    T_real = (per + P - 1) // P
    T_all = SH // P

    # --- dst ownership: ascending-degree rank, round-robin across cores ---
    order = np.argsort(deg, kind="stable")          # node ids by degree asc
    owner = np.empty(n_nodes, dtype=np.int64)
    pos = np.empty(n_nodes, dtype=np.int64)
    ranks = np.arange(n_nodes)
    owner[order] = ranks % C
    pos[order] = ranks // C
    assert pos.max() == per - 1

    # --- per-core CSR of in-edges by sorted dst position ---
    # edge e belongs to core owner[dst[e]] at position pos[dst[e]]
    e_owner = owner[dst]
    e_pos = pos[dst]
    # counts[c, p] = indegree of core c's p-th dst
    counts = np.zeros((C, per), dtype=np.int64)
    np.add.at(counts, (e_owner, e_pos), 1)

    # per-tile K: layer1 includes a self slot (table is locally replicated);
    # layer2's self-loop is a plain local DMA from the core's own shard
    K1_list, K2_list = [], []
    for t in range(T_real):
        lo, hi = t * P, min((t + 1) * P, per)
        m = int(counts[:, lo:hi].max())
        K1_list.append(m + 1)
        K2_list.append(max(m, 1))
    totK1, totK2 = sum(K1_list), sum(K2_list)
    offs1 = np.concatenate([[0], np.cumsum(K1_list)]).astype(np.int64)
    offs2 = np.concatenate([[0], np.cumsum(K2_list)]).astype(np.int64)

    # single sharding everywhere: node n lives at shard row pos(n) of core
    # owner(n); both tables use row(n) = owner*SH + pos
    row = owner * SH + pos
    pad = per  # shard pad rows are zero

    nodes_by_cp = np.full((C, per), -1, dtype=np.int64)
    nodes_by_cp[owner, pos] = np.arange(n_nodes)

    # --- build idx arrays ---
    idx1 = np.full((C, P, totK1), pad, dtype=np.int32)
    idx2 = np.full((C, P, totK2), pad, dtype=np.int32)
    # layer-1 self slot first
    for c in range(C):
        for t in range(T_real):
            lo, hi = t * P, min((t + 1) * P, per)
            sel = nodes_by_cp[c, lo:hi]
            idx1[c, : hi - lo, offs1[t]] = row[sel]
    # sort edges by (owner, pos), then fill sequentially
    eorder = np.lexsort((src, e_pos, e_owner))
    so, sp, ss = e_owner[eorder], e_pos[eorder], src[eorder]
    grp = so * per + sp
    newgrp = np.ones(len(grp), dtype=bool)
    newgrp[1:] = grp[1:] != grp[:-1]
    gstart = np.where(newgrp)[0]
    slot = np.arange(len(grp)) - np.repeat(gstart, np.diff(np.concatenate([gstart, [len(grp)]])))
    tt = sp // P                                  # tile of each edge
    lane = sp % P
    idx1[so, lane, offs1[tt] + 1 + slot] = row[ss]
    idx2[so, lane, offs2[tt] + slot] = row[ss]

    # --- dinv in sorted-shard order (per core) + full-table order ---
    dinv_s = np.zeros((C, P, T_all), dtype=np.float32)
    for c in range(C):
        fulls = np.zeros(SH, np.float32)
        fulls[:per] = dinv[nodes_by_cp[c]]
        dinv_s[c] = fulls.reshape(T_all, P).T
    dinv_f = np.concatenate(
        [dinv_s[c].T for c in range(C)], axis=0).T.copy()  # [P, C*T_all]

    meta = dict(per=per, SH=SH, T_real=T_real, T_all=T_all,
                K1_list=K1_list, K2_list=K2_list, offs1=offs1, offs2=offs2,
                totK1=totK1, totK2=totK2, nodes_by_cp=nodes_by_cp, dinv=dinv)
    return idx1, idx2, dinv_f, dinv_s, meta


def _build_nc(n_nodes, meta, phases=("p1", "l1", "ag2", "l2"),
              shared_tabs=True, reps=1, tab_bf16=False):
    phases = set(phases)
    SH, T_real, T_all = meta["SH"], meta["T_real"], meta["T_all"]
    totK1, totK2 = meta["totK1"], meta["totK2"]
    TAB = C * SH
    TG = TAB // P             # full-table 128-row groups
    TD = mybir.dt.bfloat16 if tab_bf16 else F32

    nc = bacc.Bacc("TRN2", target_bir_lowering=False, debug=False, num_devices=C)
    xT = nc.dram_tensor("xT", [D, TAB], F32, kind="ExternalInput").ap()
    w1 = nc.dram_tensor("w1", [D, H], F32, kind="ExternalInput").ap()
    w2p = nc.dram_tensor("w2p", [H, H], F32, kind="ExternalInput").ap()
    b1r = nc.dram_tensor("b1r", [P, H], F32, kind="ExternalInput").ap()
    b2r = nc.dram_tensor("b2r", [P, H], F32, kind="ExternalInput").ap()
    c2r = nc.dram_tensor("c2r", [P, H], F32, kind="ExternalInput").ap()
    ident = nc.dram_tensor("ident", [P, P], F32, kind="ExternalInput").ap()
    dinvf = nc.dram_tensor("dinvf", [P, TG], F32, kind="ExternalInput").ap()
    dinvs = nc.dram_tensor("dinvs", [P, T_all], F32, kind="ExternalInput").ap()
    idx1 = nc.dram_tensor("idx1", [P, totK1], I32, kind="ExternalInput").ap()
    idx2 = nc.dram_tensor("idx2", [P, totK2], I32, kind="ExternalInput").ap()
    out = nc.dram_tensor("out", [SH, H], F32, kind="ExternalOutput").ap()

    with tile.TileContext(nc) as tc:
        with (
            tc.tile_pool(name="cst", bufs=1) as cst,
            tc.tile_pool(name="sb", bufs=3) as sb,
            tc.tile_pool(name="gp", bufs=3) as gp,
            tc.tile_pool(name="ps", bufs=2, space="PSUM") as ps,
            tc.tile_pool(name="dram", bufs=1, space="DRAM") as dram,
        ):
            tab_space = "Shared" if shared_tabs else "Local"
            tab1 = dram.tile([TAB, H], TD)           # locally replicated
            h2s = dram.tile([SH, H], TD)
            h2f = dram.tile([SH, H], F32)
            tab2 = dram.tile([TAB, H], TD, addr_space=tab_space)

            # constants
            w1t = cst.tile([P, 4 * H], F32)
            for f in range(4):
                nc.sync.dma_start(w1t[:, f * H:(f + 1) * H],
                                  w1[f * P:(f + 1) * P, :])
            w2pt = cst.tile([H, H], F32)
            nc.sync.dma_start(w2pt[:], w2p[:, :])
            b1t = cst.tile([P, H], F32)
            nc.sync.dma_start(b1t[:], b1r[:, :])
            b2t = cst.tile([P, H], F32)
            nc.sync.dma_start(b2t[:], b2r[:, :])
            c2t = cst.tile([P, H], F32)
            nc.sync.dma_start(c2t[:], c2r[:, :])
            idt = cst.tile([P, P], F32)
            nc.sync.dma_start(idt[:], ident[:, :])
            dft = cst.tile([P, TG], F32)
            nc.sync.dma_start(dft[:], dinvf[:, :])
            dst_ = cst.tile([P, T_all], F32)
            nc.sync.dma_start(dst_[:], dinvs[:, :])
            ix1 = cst.tile([P, totK1], I32)
            nc.sync.dma_start(ix1[:], idx1[:, :])
            ix2 = cst.tile([P, totK2], I32)
            nc.sync.dma_start(ix2[:], idx2[:, :])
            ztd = cst.tile([P, H], TD)
            nc.vector.memset(ztd[:], 0.0)

            env = dict(locals())
            for _rep in range(reps):
                if _rep > 0:
                    t2r = dram.tile([TAB, H], TD, addr_space=tab_space,
                                    tag=f"tab2r{_rep}")
                    env["tab2"] = t2r
                _body(nc, tc, phases, meta, env)

    nc.compile()
    return nc


def _body(nc, tc, phases, meta, env):
    SH, T_real, T_all = meta["SH"], meta["T_real"], meta["T_all"]
    K1_list, K2_list = meta["K1_list"], meta["K2_list"]
    offs1, offs2 = meta["offs1"], meta["offs2"]
    TAB = C * SH
    NSTF = TAB // 512          # P1 supertiles over the FULL table
    maxK1, maxK2 = max(K1_list), max(K2_list)
    TD = env["TD"]
    xT = env["xT"]; out = env["out"]
    sb = env["sb"]; gp = env["gp"]; ps = env["ps"]
    tab1 = env["tab1"]; h2s = env["h2s"]; h2f = env["h2f"]; tab2 = env["tab2"]
    w1t = env["w1t"]; w2pt = env["w2pt"]; b1t = env["b1t"]; b2t = env["b2t"]
    c2t = env["c2t"]; idt = env["idt"]; dft = env["dft"]; dst_ = env["dst_"]
    ix1 = env["ix1"]; ix2 = env["ix2"]; ztd = env["ztd"]

    # ---- P1 (replicated): every core computes the FULL table1 ----
    for st in range(NSTF if "p1" in phases else 0):
        xt = sb.tile([P, 4 * D], F32, tag="xt")  # 4 feat chunks x 512 rows
        for f in range(4):
            nc.sync.dma_start(
                xt[:, f * D:(f + 1) * D],
                xT[f * P:(f + 1) * P, st * 512:(st + 1) * 512])
        for g4 in range(4):
            pp = ps.tile([P, H], F32, tag="p1ps")
            for f in range(4):
                nc.tensor.matmul(
                    pp[:],
                    lhsT=xt[:, f * D + g4 * P: f * D + (g4 + 1) * P],
                    rhs=w1t[:, f * H:(f + 1) * H],
                    start=(f == 0), stop=(f == 3))
            g = st * 4 + g4
            ht = sb.tile([P, H], TD, tag="ht")
            nc.scalar.activation(ht[:], pp[:],
                                 mybir.ActivationFunctionType.Copy,
                                 scale=dft[:, g:g + 1])
            nc.sync.dma_start(tab1[g * P:(g + 1) * P, :], ht[:])

    # zero pad rows of h2s (tiles >= T_real never written)
    if "p1" in phases:
        for t in range(T_real, T_all):
            nc.sync.dma_start(h2s[t * P:(t + 1) * P, :], ztd[:])

    # ---- Layer 1 aggregation + epilogue (self-loop is idx slot 0) ----
    for t in range(T_real if "l1" in phases else 0):
        K = K1_list[t]
        g = gp.tile([P, maxK1 * H], TD, tag="g1")
        for j in range(K):
            nc.gpsimd.indirect_dma_start(
                out=g[:, j * H:(j + 1) * H], out_offset=None,
                in_=tab1[:],
                in_offset=bass.IndirectOffsetOnAxis(
                    ap=ix1[:, offs1[t] + j: offs1[t] + j + 1], axis=0))
        red = sb.tile([P, H], F32, tag="red")
        nc.vector.reduce_sum(
            out=red[:],
            in_=g[:, :K * H].rearrange("p (j f) -> p f j", f=H),
            axis=mybir.AxisListType.X)
        nc.vector.tensor_scalar_mul(red[:], red[:], dst_[:, t:t + 1])
        nc.vector.tensor_add(red[:], red[:], b1t[:])
        nc.vector.tensor_scalar_max(red[:], red[:], 0.0)
        pt = ps.tile([H, P], F32, tag="pst")
        nc.tensor.transpose(pt[:], red[:], idt[:])
        rt = sb.tile([H, P], F32, tag="rt")
        nc.scalar.activation(rt[:], pt[:],
                             mybir.ActivationFunctionType.Copy)
        p2 = ps.tile([P, H], F32, tag="ps2")
        nc.tensor.matmul(p2[:], lhsT=rt[:], rhs=w2pt[:],
                         start=True, stop=True)
        h2ff = sb.tile([P, H], F32, tag="h2ff")
        nc.vector.tensor_add(h2ff[:], p2[:], c2t[:])
        nc.vector.tensor_scalar_mul(h2ff[:], h2ff[:], dst_[:, t:t + 1])
        nc.sync.dma_start(h2f[t * P:(t + 1) * P, :], h2ff[:])
        h2t = sb.tile([P, H], TD, tag="h2t")
        nc.vector.tensor_copy(h2t[:], h2ff[:])
        nc.sync.dma_start(h2s[t * P:(t + 1) * P, :], h2t[:])

    # ---- AllGather 2 ----
    if "ag2" in phases:
        nc.gpsimd.collective_compute(
            "AllGather", mybir.AluOpType.bypass,
            replica_groups=[list(range(C))],
            ins=[h2s.opt()], outs=[tab2.opt()])

    # ---- Layer 2 aggregation + epilogue (self via local f32 DMA) ----
    for t in range(T_real if "l2" in phases else 0):
        K = K2_list[t]
        g = gp.tile([P, maxK2 * H], TD, tag="g2")
        for j in range(K):
            nc.gpsimd.indirect_dma_start(
                out=g[:, j * H:(j + 1) * H], out_offset=None,
                in_=tab2[:],
                in_offset=bass.IndirectOffsetOnAxis(
                    ap=ix2[:, offs2[t] + j: offs2[t] + j + 1], axis=0))
        sf = sb.tile([P, H], F32, tag="sf2")
        nc.sync.dma_start(sf[:], h2f[t * P:(t + 1) * P, :])
        red = sb.tile([P, H], F32, tag="red2")
        nc.vector.reduce_sum(
            out=red[:],
            in_=g[:, :K * H].rearrange("p (j f) -> p f j", f=H),
            axis=mybir.AxisListType.X)
        nc.vector.tensor_add(red[:], red[:], sf[:])
        nc.vector.tensor_scalar_mul(red[:], red[:], dst_[:, t:t + 1])
        nc.vector.tensor_add(red[:], red[:], b2t[:])
        nc.vector.tensor_scalar_max(red[:], red[:], 0.0)
        ot = sb.tile([P, H], F32, tag="ot")
        nc.vector.tensor_copy(ot[:], red[:])
        nc.sync.dma_start(out[t * P:(t + 1) * P, :], ot[:])


def _impl(x, edge_index, W1, b1, W2, b2, gamma, beta, run_mean, run_var,
          n_nodes):
    x = np.asarray(x, np.float32)
    W1 = np.asarray(W1, np.float32)
    b1 = np.asarray(b1, np.float32)
    W2 = np.asarray(W2, np.float32)
    b2 = np.asarray(b2, np.float32)
    gamma = np.asarray(gamma, np.float32)
    beta = np.asarray(beta, np.float32)
    run_mean = np.asarray(run_mean, np.float32)
    run_var = np.asarray(run_var, np.float32)

    idx1, idx2, dinv_f, dinv_s, meta = _plan(n_nodes, np.asarray(edge_index))
    per, SH = meta["per"], meta["SH"]

    # BN folding
    s = gamma / np.sqrt(run_var + BN_EPS)
    t = beta - run_mean * s
    W2p = (W2 * s[:, None]).astype(np.float32)
    c2 = (t @ W2).astype(np.float32)

    b1rep = np.tile(b1[None, :], (P, 1)).astype(np.float32)
    b2rep = np.tile(b2[None, :], (P, 1)).astype(np.float32)
    c2rep = np.tile(c2[None, :], (P, 1)).astype(np.float32)
    identv = np.eye(P, dtype=np.float32)

    nodes_by_cp = meta["nodes_by_cp"]
    # full x in table order (all shards concatenated), same for every core
    xs = np.zeros((C * SH, D), np.float32)
    for c in range(C):
        xs[c * SH:c * SH + per] = x[nodes_by_cp[c]]
    xTfull = np.ascontiguousarray(xs.T)
    in_maps = []
    for c in range(C):
        in_maps.append({
            "xT": xTfull, "w1": W1, "w2p": W2p, "b1r": b1rep, "b2r": b2rep,
            "c2r": c2rep, "ident": identv,
            "dinvf": np.ascontiguousarray(dinv_f),
            "dinvs": np.ascontiguousarray(dinv_s[c]),
            "idx1": np.ascontiguousarray(idx1[c]),
            "idx2": np.ascontiguousarray(idx2[c]),
        })

    nc = _build_nc(n_nodes, meta)
    global _LAST_NC, _LAST_IN_MAPS, _LAST_META
    _LAST_NC, _LAST_IN_MAPS, _LAST_META = nc, in_maps, meta
    res = run_bass_kernel_spmd(nc, in_maps, core_ids=list(range(C))).results

    outf = np.zeros((n_nodes, H), np.float32)
    for c in range(C):
        outf[nodes_by_cp[c]] = res[c]["out"][:per]
    return outf


def kernel(x, edge_index, W1, b1, W2, b2, gamma, beta, run_mean, run_var):
    return _impl(x, edge_index, W1, b1, W2, b2, gamma, beta, run_mean,
                 run_var, n_nodes=100000)

